# revision 19
# baseline (speedup 1.0000x reference)
"""V9: balanced bins, zero empty gather calls, static full-call counts.

V6 sized every group for the worst case (16 calls/group = 64 calls/core)
while typical data fills ~34 calls; each of the ~30 empty calls cost
~1.7us of serialized Pool/SWDGE latency (994ns fixed gen + 650ns DGE
delay with no transfer to hide under) = ~50us of DMA idle per core
(TimelineSim: 258.8us total vs 201.4us DMA busy).

V7-V9 assign b's to the 32 (core, group) bins directly (serpentine over
descending length, then LPT bins->cores), so all bins carry ~equal rows
(spread ~25 of ~8465, cores within 8 rows), and size the program to the
actual data: per group, 8 full 1024-row calls plus one short last call
(3 tiles for the fixed input seed). Every call carries data; SWDGE
generation pipelines under the previous call's transfer (mid-stream DMA
gaps are zero). Calls whose capacity lies below the smallest bin are
issued with a STATIC full count (no value_load; host pads if a bin ever
falls short); only the last call per group reads its count at runtime.
Const loads are split per group across both HWDGE engines (cnt on
Activation, call-0's idx chunk first on SP) so the first gather's deps
land in ~1us; iota is generated on-chip. Keep the call sizes
[8,...,8,small]: tapering into several small calls fragments the PE
stream and drops it to the mid p-state (427ns vs 213ns per matmul),
which lands on the critical flush chain (costs ~3us, measured).

TimelineSim: 206.4us/core vs 196.7us DMA busy (95.3%) — within ~5% of
the gather-bytes roofline (33.9k rows * 2KiB / 360GB/s = 192.8us).

V10 post-mortem (tried, reverted): a compile-time positional first call
(static idx built on-chip via iota + [16->128] replication matmul,
gathering rows [0,128) of the first K0 slots) moves the first DMA byte
from 5.0us to 3.8us, but the extra K0-matmul burst at every group
boundary perturbs the PE p-state rhythm: ~half of all matmul bursts
run cold (788ns vs 213ns) in any schedule, and with the static call
the cold phases land on the critical path (+1.7us mid-stream), netting
+0.6us across every tail shape tried. Keep the uniform [8,...,8,small]
rhythm.

Core compute (unchanged from V6): exact ragged gather of seq rows in
[begin,end) via gpsimd.dma_gather (int16 idx, runtime valid-count),
per row-tile a [128,64] selection matrix built on DVE in one
tensor_scalar (sel[k,j] = (colidx[k]==j) * 1/len), TensorE f32r
matmul psum[64,512] += sel.T @ gtile accumulated over the group's
tiles, flushed to outn[grp*64:(grp+1)*64].
"""

import time

import numpy as np

import concourse.bass as bass
from concourse import bacc
import concourse.mybir as mybir
import concourse.tile as tile
from concourse.bass_utils import run_bass_kernel_spmd

B, L, D = 2048, 512, 512
NCORES = 8
BL = B // NCORES  # 256
GB = 64  # b's per group (region = GB*L = 32768 rows, int16 idx max)
CT = 8  # tiles per full dma_gather call (8*128=1024 idx = SWDGE ring cap)
GRPS = BL // GB  # 4 groups per core
NBINS = NCORES * GRPS  # 32
BUFS = 6  # gtile pool depth; first BUFS calls must be fully written

_CACHE = {}
LAST_RESULTS = None
LAST_SPMD = None
STATIC_CNTS = None  # tlsim-only: per-call static num_idxs specialization
RACE_CHECK = True


def _build_bass(ctiles, static_full):
    """ctiles: per-group call sizes in row-tiles. static_full: per-call flag,
    True -> the call always carries its full capacity (static count)."""
    nc = bacc.Bacc("TRN2", detect_race_conditions=RACE_CHECK)
    f32 = mybir.dt.float32
    i32 = mybir.dt.int32
    f32r = mybir.dt.float32r
    ncalls = len(ctiles)
    nt = sum(ctiles)  # tiles per group
    gcols = nt * 8  # idx columns per group (16 idx per column)
    c0cols = ctiles[0] * 8
    seq = nc.dram_tensor("seq", [BL, L, D], f32r, kind="ExternalInput")
    # per-tile selection inputs: columns 2t = colidx, 2t+1 = w  (GRPS*nt tiles)
    colw = nc.dram_tensor("colw", [128, GRPS * nt * 2], f32, kind="ExternalInput")
    gidx = nc.dram_tensor("gidx", [128, GRPS * gcols], mybir.dt.int16,
                          kind="ExternalInput")
    gcnt = nc.dram_tensor("gcnt", [1, GRPS * ncalls], i32, kind="ExternalInput")
    outn = nc.dram_tensor("outn", [BL, D], f32, kind="ExternalOutput")

    rows = seq[:].rearrange("b l d -> (b l) d")  # [BL*L, D]

    with tile.TileContext(nc) as tc:
        with (
            tc.tile_pool(name="gpool", bufs=BUFS) as gpool,
            tc.tile_pool(name="selp", bufs=6) as selp,
            tc.tile_pool(name="constp", bufs=1) as constp,
            tc.tile_pool(name="psump", bufs=2, space="PSUM") as psump,
            tc.tile_pool(name="outp", bufs=2) as outp,
        ):
            cnt_sb = constp.tile([1, GRPS * ncalls], i32)
            idx_sb = []
            colw_sb = []
            for grp in range(GRPS):
                idx_sb.append(constp.tile([128, gcols], mybir.dt.int16,
                                          name=f"idx{grp}"))
                colw_sb.append(constp.tile([128, nt * 2], f32,
                                           name=f"colw{grp}"))
            iota_f = constp.tile([128, GB], f32)
            # call 0's generation gates only on its idx chunk (static count):
            # issue it first on SP; cnt and the rest stream on both HWDGE
            # queues under the gathers
            nc.sync.dma_start(out=idx_sb[0][:, 0:c0cols], in_=gidx[:, 0:c0cols])
            nc.scalar.dma_start(out=cnt_sb[:], in_=gcnt[:])
            nc.sync.dma_start(out=idx_sb[0][:, c0cols:gcols],
                              in_=gidx[:, c0cols:gcols])
            nc.gpsimd.iota(
                out=iota_f[:],
                pattern=[[1, GB]],
                base=0,
                channel_multiplier=0,
                allow_small_or_imprecise_dtypes=True,
            )
            nc.scalar.dma_start(out=colw_sb[0][:], in_=colw[:, 0 : nt * 2])
            for grp in range(1, GRPS):
                nc.sync.dma_start(
                    out=idx_sb[grp][:],
                    in_=gidx[:, grp * gcols : (grp + 1) * gcols],
                )
                nc.scalar.dma_start(
                    out=colw_sb[grp][:],
                    in_=colw[:, grp * nt * 2 : (grp + 1) * nt * 2],
                )

            # no memsets (ISA rejects f32r memset): the first BUFS calls are
            # full (host prep pads short bins), so every physical gather
            # slot is fully written before any stale-region read
            for grp in range(GRPS):
                psum = psump.tile([GB, D], f32, tag="ps", name="psum")
                tbase = 0  # tile id within group
                cbase = 0  # idx column within group
                for call in range(ncalls):
                    ct_c = ctiles[call]
                    g = grp * ncalls + call
                    gtile = gpool.tile([128, ct_c * D], f32r, tag="g",
                                       name="gtile")
                    if STATIC_CNTS is None:
                        if static_full[call]:
                            cnt_rv = ct_c * 128
                        else:
                            # no min/max: the runtime assert they emit wedges
                            # the device under this runtime
                            cnt_rv = nc.gpsimd.value_load(cnt_sb[0:1, g : g + 1])
                        nc.gpsimd.dma_gather(
                            gtile[:].rearrange("p (c e) -> p c e", e=D),
                            rows[grp * GB * L : (grp + 1) * GB * L, :],
                            idx_sb[grp][:, cbase : cbase + ct_c * 8],
                            ct_c * 128,
                            cnt_rv,
                            D,
                        )
                    else:
                        cnt = int(STATIC_CNTS[g])
                        ni = -(-cnt // 16) * 16  # round up to 16
                        nc.gpsimd.dma_gather(
                            gtile[:].rearrange("p (c e) -> p c e", e=D)[
                                :, : -(-ni // 128), :
                            ],
                            rows[grp * GB * L : (grp + 1) * GB * L, :],
                            idx_sb[grp][:, cbase : cbase + ni // 16],
                            ni,
                            cnt,
                            D,
                        )
                    for t in range(ct_c):
                        tg = tbase + t  # tile id within group
                        sel = selp.tile([128, GB], f32r, tag="sel", name="sel")
                        nc.vector.tensor_scalar(
                            out=sel[:],
                            in0=iota_f[:],
                            scalar1=colw_sb[grp][:, 2 * tg : 2 * tg + 1],
                            scalar2=colw_sb[grp][:, 2 * tg + 1 : 2 * tg + 2],
                            op0=mybir.AluOpType.is_equal,
                            op1=mybir.AluOpType.mult,
                        )
                        tile_first = call == 0 and t == 0
                        tile_last = call == ncalls - 1 and t == ct_c - 1
                        nc.tensor.matmul(
                            out=psum[:],
                            lhsT=sel[:],
                            rhs=gtile[:, t * D : (t + 1) * D],
                            start=tile_first,
                            stop=tile_last,
                        )
                    tbase += ct_c
                    cbase += ct_c * 8
                out_sb = outp.tile([GB, D], f32, tag="out", name="out_sb")
                nc.vector.tensor_copy(out=out_sb[:], in_=psum[:])
                nc.sync.dma_start(
                    out=outn[grp * GB : (grp + 1) * GB, :], in_=out_sb[:]
                )
    nc.compile()
    return nc


def _get_bass(ctiles, static_full):
    key = ("nc", tuple(ctiles), tuple(static_full))
    if key not in _CACHE:
        _CACHE[key] = _build_bass(ctiles, static_full)
    return _CACHE[key]


def _host_prep(begin_c, end_c, ctiles, static_full):
    """Compacted per-group gather indices, per-call counts, per-tile col/w."""
    length = (end_c - begin_c).astype(np.int64)
    w_b = 1.0 / length.astype(np.float32)
    ncalls = len(ctiles)
    nt = sum(ctiles)  # tiles per group
    rows_cap = nt * 128  # row capacity per group
    offs = np.concatenate([[0], np.cumsum([c * 128 for c in ctiles])])
    ncalls_c = GRPS * ncalls  # calls per core
    idx_all = np.full((GRPS * rows_cap,), -1, dtype=np.int64)
    colidx = np.full((GRPS * nt, 128), -1.0, dtype=np.float32)
    wcol = np.zeros((GRPS * nt, 128), dtype=np.float32)
    cnt = np.zeros(ncalls_c, dtype=np.int32)
    for grp in range(GRPS):
        bs = np.arange(grp * GB, (grp + 1) * GB)
        lens = length[bs]
        n_rows = int(lens.sum())
        assert n_rows <= rows_cap, (n_rows, rows_cap)
        # stream of (slot, l) for all rows of the group, in slot order
        slots = np.repeat(np.arange(GB), lens)
        ls = np.concatenate([np.arange(begin_c[b], end_c[b]) for b in bs])
        ridx = slots * L + ls  # row index within group region
        base = grp * rows_cap
        idx_all[base : base + n_rows] = ridx
        tiles = np.arange(n_rows) // 128
        pos = np.arange(n_rows) % 128
        colidx[grp * nt + tiles, pos] = slots.astype(np.float32)
        wcol[grp * nt + tiles, pos] = w_b[bs][slots]
        for call in range(ncalls):
            cap = ctiles[call] * 128
            c = min(max(n_rows - int(offs[call]), 0), cap)
            g = grp * ncalls + call
            if static_full[call] or g < BUFS:
                # static-count calls always transfer full capacity, and the
                # first BUFS gather slots must be fully written on first use
                # (boot NaN guard): pad tail with row 0 — padding rows have
                # colidx -1 -> zero selection
                if c < cap:
                    sl_ = slice(base + int(offs[call]) + c,
                                base + int(offs[call + 1]))
                    idx_all[sl_] = 0
                    c = cap
            elif c == 0:
                # avoid fully-empty calls (sim chokes; HW gains nothing)
                idx_all[base + int(offs[call])] = 0
                c = 1
            cnt[g] = c
    assert idx_all.max() < GB * L
    idx16 = idx_all.astype(np.int16).reshape(-1, 16).T  # [16, total/16]
    idx = np.ascontiguousarray(np.tile(idx16, (8, 1)))  # [128, total/16]
    # colw[p, 2t] = colidx, colw[p, 2t+1] = w
    colw = np.empty((128, GRPS * nt * 2), dtype=np.float32)
    colw[:, 0::2] = colidx.T
    colw[:, 1::2] = wcol.T
    cnt2 = cnt.reshape(1, ncalls_c)
    return np.ascontiguousarray(colw), idx, np.ascontiguousarray(cnt2)


def _balanced_assignment(length, begin_end=None):
    """Assign b's to the NBINS (core, group) bins, serpentine over
    descending length, so per-bin total gathered rows (the DMA-bound cost)
    are near-equal. Returns [NCORES, BL]: bin k = core k//GRPS, group
    k%GRPS, i.e. rows [GB*(k%GRPS) : GB*(k%GRPS+1)] of core k//GRPS."""
    order = np.argsort(-length, kind="stable")
    bins = np.empty((NBINS, GB), dtype=np.int64)
    for r in range(GB):
        idxs = range(NBINS) if r % 2 == 0 else range(NBINS - 1, -1, -1)
        for j, k in enumerate(idxs):
            bins[k, r] = order[r * NBINS + j]
    # assign bins to cores LPT-style so per-core totals (4 bins each) are
    # flat, not just per-bin totals
    bin_rows = length[bins].sum(axis=1)
    by_rows = np.argsort(-bin_rows, kind="stable")
    core_rows = np.zeros(NCORES, dtype=np.int64)
    core_fill = np.zeros(NCORES, dtype=np.int64)
    perm = np.empty(NBINS, dtype=np.int64)
    for k in by_rows:
        c = int(np.argmin(np.where(core_fill < GRPS, core_rows, np.iinfo(np.int64).max)))
        perm[c * GRPS + core_fill[c]] = k
        core_rows[c] += bin_rows[k]
        core_fill[c] += 1
    return bins[perm].reshape(NCORES, BL)


def _plan_calls(length, asm, begin_end=None):
    """Static per-group call plan: full 1024-row calls followed by a small
    taper ([4, 4, <=7] tiles) so the final PE/flush chain after the last
    gather is short. Returns (ctiles, static_full): call sizes in tiles and
    per-call static-count flags (True where capacity fits above the
    smallest bin, so the call is always full)."""
    bin_rows = length[asm.reshape(NBINS, GB)].sum(axis=1)
    mx = int(bin_rows.max())
    mn = int(bin_rows.min())
    tiles_needed = max(-(-mx // 128), 1)
    nfull = tiles_needed // CT
    rem = tiles_needed - nfull * CT
    # NOTE: do not taper the tail into several small calls — fragmented PE
    # bursts drop the Tensor engine to the mid p-state (427ns vs 213ns per
    # matmul after 3us of continuous work), which lands on the critical
    # flush chain and costs more than the shorter last call saves
    if rem == 0:
        nfull -= 1
        rem = CT
    ctiles = [CT] * nfull + [rem]
    offs = np.cumsum([c * 128 for c in ctiles])
    static_full = [int(o) <= mn for o in offs]
    return ctiles, static_full


def kernel(seq, begin, end):
    global LAST_RESULTS, LAST_SPMD
    seq = np.ascontiguousarray(np.asarray(seq, dtype=np.float32))
    begin_i = np.asarray(begin).astype(np.int64)
    end_i = np.asarray(end).astype(np.int64)
    length = end_i - begin_i
    asm = _balanced_assignment(length)
    ctiles, static_full = _plan_calls(length, asm)

    nc = _get_bass(ctiles, static_full)
    in_maps = []
    for c in range(NCORES):
        bs = asm[c]
        colw, idx, cnt = _host_prep(begin_i[bs], end_i[bs], ctiles, static_full)
        in_maps.append({"seq": seq[bs], "colw": colw, "gidx": idx, "gcnt": cnt})

    LAST_SPMD = (nc, in_maps)
    # the axon-tunneled devices occasionally report a transient
    # NRT_EXEC_UNIT_UNRECOVERABLE; a fresh attempt recovers
    last_exc = None
    for attempt in range(3):
        try:
            LAST_RESULTS = run_bass_kernel_spmd(
                nc, in_maps, core_ids=list(range(NCORES))
            )
            break
        except Exception as e:  # noqa: BLE001
            last_exc = e
            time.sleep(10.0)
    else:
        raise last_exc
    out = np.empty((B, D), dtype=np.float32)
    for c in range(NCORES):
        out[asm[c]] = LAST_RESULTS.results[c]["outn"]
    return out
